# revision 63
# baseline (speedup 1.0000x reference)
"""Trainium2 Bass kernel for a 2-state linear-chain CRF loss (BiLSTM-CRF loss_fn).

Computes, for a single conversation of length T = 2,097,152:
  gold_score  = sum_t em[t, lab[t]] + sum_{t>0} trans[t][lab[t-1], lab[t]]
  total_score = logsumexp of the CRF forward recursion
where trans[t] = who2who_sub[w[t]] + position_sub[p[t]] (19 position + 2
who2who matrices; indices 19/2 select an all-zero padding matrix).

Design (one NeuronCore per contiguous chunk of 262,144 steps, 8 cores):

* Per-step matrices M[t][i,j] = trans[t][i,j] + em[t][j] are built by
  per-class masked accumulation: each (class, comp) is one fused fp16
  tensor_scalar mv = (idx == c) * V  (fast 4x 2-byte DVE mode).  The
  accumulation adds -- the expensive half at 2x -- are split across THREE
  sinks that run in parallel: DVE tensor_adds into ACC, GPSIMD tensor_adds
  into a second accumulator, and SBUF->SBUF *accumulate DMAs* (software-DGE
  cce add) that fold whole 4-comp mv tiles into two subaccumulator chains
  on the otherwise-idle DMA engines.  ACC init = emission columns (ACT Copy),
  so the em fold costs nothing extra.

* Gold score: gold = sum_t M[t][lab[t-1], lab[t]], computed by three
  copy_predicated selects (lab / labp as masks) directly on the finished
  ACC comps (in place, after the tree's level-1 reads), then one fused
  reduce.  fp16 value-rounding bias measured ~5e-4 rel -- far inside the
  tolerance, and 10x cheaper than exact per-cell counting.

* Forward pass: the recursion is a product of 2x2 matrices in the (log,+)
  semiring (associative).  The host ships every per-partition stream in
  BIT-REVERSED step order, so each of the 3 on-device tree levels combines
  the first half of a tile with the second half -- fully contiguous fp16
  operands at 2x, no strided gathers.  LSE(a,b) = a + ln(1+exp(b-a)) with
  the exp/ln intermediate in f32 (fp16 exp would overflow past d~11).
  The device stops at 256 matrices per partition (8 source steps each);
  the host finishes the remaining 18 tree levels vectorized in numpy
  (O(cores*partitions) work, independent of T).

* All inputs ship as a single per-core fp16 blob
  [par | p | w | em0 | em1 | lab | labp] with em0/em1 stored as separate
  contiguous planes so every device op reads packed rows.

Accuracy vs the fp32 jax reference: gold ~5e-4 rel; total ~1e-3 rel (the
reference's own sequential-fp32-scan rounding wander at T=2M).
"""

from contextlib import ExitStack

import numpy as np

import concourse.bass as bass
import concourse.bacc as bacc
import concourse.mybir as mybir
import concourse.tile as tile
from concourse import bass_utils

dt = mybir.dt
ALU = mybir.AluOpType
AF = mybir.ActivationFunctionType
AX = mybir.AxisListType

T = 2097152
NCORES = 8
P = 128                  # SBUF partitions
L = T // NCORES          # steps per core = 262144
F = L // P               # steps per partition = 2048
H = F // 2               # pairs per partition at tree level 1
WSTOP = 512              # matrices per partition shipped to the host
NPOS = 19                # position classes with nonzero matrices
# param row layout (f32 columns):
#   0..79    W_pos  = pos_param + B for 20 classes (incl. pad 19 -> B)
#   80..87   V_w2w  (unshifted)
#   88       -B     (em-init bias; cancels the one +B every element gets)
#   89..168  -W_pos (ACT Relu scale operands)
NPAR = 169
# blob (fp16): [par | p | w | emP0..emP3 | lab | labp]; emPc = em_{c&1} - B
# + a_c with a_c the constant term of the who2who quadratic for comp c
W0 = 8 * F + 2 * NPAR

# class-sum routing: 22 classes total (20 position incl. pad + 2 who2who).
# N_R2 classes ride the DMA-accumulate chains (the first N_ACT of them get
# their masked mv built on the ACT engine via Relu(W - W*(p-c)^2)); one
# class writes its mv directly into the GPSIMD accumulator (zero adds); the
# remaining classes' comp-adds are split N_GPADD to GPSIMD, rest to DVE.
N_R2 = 12
N_ACT = 0
N_GPADD = 8
N_R4 = 1  # kept for tile guards (the direct-write accP class)
MV4_BUFS = 3

# debug switches (bisect aids; all True for the real kernel)
EN_TREE = True
EN_GOLD = True
EN_LEV23 = True

_NC_CACHE = None
LAST_RESULTS = None  # BassKernelResults of the most recent kernel() call


def _comp(i, j):
    return i * 2 + j


def _build_nc():
    nc = bacc.Bacc()

    b0_d = nc.dram_tensor("blob0", [P, W0], dt.float16, kind="ExternalInput")
    outm_d = nc.dram_tensor("outm", [P, 4 * WSTOP], dt.float16,
                            kind="ExternalOutput")
    outg_d = nc.dram_tensor("outg", [P, 1], dt.float32, kind="ExternalOutput")

    # const APs for the ACT-route Square bias values (-class id); the
    # ACT-fed classes sit at odd chain positions
    for _v in sorted({-float(2 * k + 1) for k in range(N_ACT)}):
        if (dt.float32, _v) in nc.const_aps.aps:
            continue
        _t = nc.alloc_sbuf_tensor(f"const-float32-{_v}", [128, 1], dt.float32)
        nc.gpsimd.memset(_t.ap(), _v)
        nc.const_aps.aps[(dt.float32, _v)] = _t.ap()
    nc.all_engine_barrier()

    with ExitStack() as ctx:
        tc = ctx.enter_context(tile.TileContext(nc))
        pool = ctx.enter_context(tc.tile_pool(name="main", bufs=1))
        ppool = ctx.enter_context(tc.psum_pool(name="psum", bufs=1))

        # ---- loads ----
        # blob layout (fp16 cols): [par | p | w | em0 | em1 | lab | labp]
        b0 = pool.tile([P, W0], dt.float16, tag="b0", name="b0")
        parw = 2 * NPAR
        o_p = parw
        o_w = o_p + F
        o_e0 = o_w + F
        o_lab = o_e0 + 4 * F
        o_labp = o_lab + F
        # head1: par|p|w (needed by every mask op), head2: emissions,
        # tail: labels (needed only by gold, emitted mid-chain below)
        nc.sync.dma_start(b0[:, 0:o_w], b0_d[:, 0:o_w])
        nc.sync.dma_start(b0[:, o_w:o_e0], b0_d[:, o_w:o_e0])
        nc.sync.dma_start(b0[:, o_e0:o_lab], b0_d[:, o_e0:o_lab])

        par32 = b0[:, 0:parw].bitcast(dt.float32)
        p_t = b0[:, o_p:o_p + F]
        w_t = b0[:, o_w:o_w + F]
        emP = [b0[:, o_e0 + c * F:o_e0 + (c + 1) * F] for c in range(4)]
        # labels ship as int16 {0,1} in the fp16 blob slots (CopyPredicated
        # requires an integer mask dtype)
        lab16 = b0[:, o_lab:o_lab + F].bitcast(dt.int16)
        labp16 = b0[:, o_labp:o_labp + F].bitcast(dt.int16)

        def V(col):
            return par32[:, col:col + 1]

        # ---- accumulators ----
        # ACC  : DVE-adds sink, init = emission columns (M[i,j] = trans+em[j])
        # accP : GPSIMD-adds sink, init = first R4 class's mv (direct ts write)
        # S0/S1: DMA-accumulate chains, init = first hop is a plain dma copy
        # S/mv4 rows are padded to F+8 so the comp dim cannot merge with the
        # row dim during DMA lowering: per-partition contiguous descriptor
        # chunks stay at 4KB (16KB single-descriptor Pool DMAs fail at
        # runtime on this stack).
        FP = F + 8
        ACC = pool.tile([P, 4, F], dt.float16, tag="ACC", name="ACC")
        S0 = (pool.tile([P, 4, FP], dt.float16, tag="S0", name="S0")
              if N_R2 > 0 else None)
        S1 = (pool.tile([P, 4, FP], dt.float16, tag="S1", name="S1")
              if N_R2 > 0 else None)
        # tree tiles allocated up front; XY1 doubles as a class-phase mv
        # buffer (idle until the tree) and SPa lives in PSUM
        XY1 = pool.tile([P, 8, H], dt.float16, tag="XY1", name="XY1")
        XY2 = pool.tile([P, 8, H // 2], dt.float16, tag="XY2", name="XY2")
        SPa = ppool.tile([P, 4, H], dt.float32, tag="SPa", name="SPa")
        SPL = pool.tile([P, 4, H], dt.float16, tag="SPL", name="SPL")
        # emission planes ship pre-shifted (em - B + a_c) on the host, so
        # plain copies init ACC
        for c in range(4):
            nc.scalar.activation(ACC[:, c, :], emP[c], AF.Copy)

        # classes: (src, cval, vcol). position classes only (incl. pad iff
        # the shifted ACT route is active); who2who is handled by the exact
        # quadratic in w below (pad w=2 -> 0 by construction).
        npos_cls = NPOS + (1 if N_ACT > 0 else 0)
        classes = [(p_t, float(c), 4 * c) for c in range(npos_cls)]
        r2_classes = classes[:N_R2]
        direct_classes = classes[N_R2:N_R2 + 2]  # init S0 and S1 (no adds)
        rest_classes = classes[N_R2 + 2:]

        mv4 = [
            pool.tile([P, 4, FP], dt.float16, tag=f"mv4_{i}", name=f"mv4_{i}")
            for i in range(MV4_BUFS if N_R2 > 0 else 0)
        ]
        # one GPSIMD mv buffer (the slow GP adds serialize anyway; cap
        # N_GPADD at 2) and two DVE mv buffers, the second aliasing XY1's
        # memory, which sits idle until the tree starts
        mv_d = [
            pool.tile([P, 4, F], dt.float16, tag="mvd0", name="mvd0"),
            pool.tile([P, 4, F], dt.float16, tag="mvd1", name="mvd1"),
        ]

        # direct-write classes: their mv IS the chain init (no adds at
        # all), so every chain hop accumulates instead of copying
        for S, (src, cval, col) in zip((S0, S1), direct_classes):
            for c in range(4):
                nc.vector.tensor_scalar(
                    S[:, c, 0:F], src, cval, V(col + c),
                    ALU.is_equal, ALU.mult,
                )

        dve_cls = rest_classes

        def emit_dve_cls(k):
            src, cval, col = dve_cls[k]
            m = mv_d[k % 2]
            for c in range(4):
                nc.vector.tensor_scalar(
                    m[:, c, :], src, cval, V(col + c), ALU.is_equal, ALU.mult
                )
            nc.vector.tensor_add(
                ACC[:].rearrange("p c f -> p (c f)"),
                ACC[:].rearrange("p c f -> p (c f)"),
                m[:].rearrange("p c f -> p (c f)"),
            )

        # Interleave: per R2 class emit its 4 fused-ts + one chain hop, then
        # a pro-rata slice of the GPSIMD classes (keeps the Pool queue
        # alternating swdge preps with adds so the chains never starve) and
        # of the DVE classes (keeps DVE busy while the chains drain).
        n_dv = len(dve_cls)
        dv_k = 0
        tsq = [
            pool.tile([P, F], dt.float16, tag=f"tsq{i}", name=f"tsq{i}")
            for i in range(2 if N_ACT > 0 else 0)
        ]
        # ACT-fed chain classes sit at odd positions so the DVE-fed hops
        # interleave and the chains never wait on the slower ACT producer
        act_ri = {2 * k + 1 for k in range(N_ACT)}
        for ri, (src, cval, col) in enumerate(r2_classes):
            m4 = mv4[ri % MV4_BUFS]
            if ri in act_ri:
                # ACT-built mv: t = (p-c)^2 ; mv_k = Relu(W_k - W_k*t)
                t = tsq[ri % 2]
                nc.scalar.activation(t[:], src, AF.Square, bias=-cval)
                for c in range(4):
                    nc.scalar.activation(
                        m4[:, c, 0:F], t[:], AF.Relu,
                        bias=V(col + c), scale=V(89 + col + c),
                    )
            else:
                for c in range(4):
                    nc.vector.tensor_scalar(
                        m4[:, c, 0:F], src, cval, V(col + c),
                        ALU.is_equal, ALU.mult,
                    )
            S = S0 if ri % 2 == 0 else S1
            nc.gpsimd.dma_start(S[:, :, 0:F], m4[:, :, 0:F],
                                accum_op=ALU.add)
            for _ in range((n_dv * (ri + 1)) // N_R2 - (n_dv * ri) // N_R2):
                emit_dve_cls(dv_k)
                dv_k += 1
            if ri == 5:
                # labels, needed late (gold) -- emitted here so the head
                # DMAs and early chain hops aren't delayed
                nc.sync.dma_start(b0[:, o_lab:W0], b0_d[:, o_lab:W0])
        while dv_k < n_dv:
            emit_dve_cls(dv_k)
            dv_k += 1

        # ---- who2who via the exact quadratic a_c + b_c*w + c_c*w^2 ----
        # (a_c folded into the emission planes; b at cols 80..83, c at 84..87)
        qt = mv_d[0]
        for c in range(4):
            nc.vector.tensor_scalar(
                qt[:, c, :], w_t, V(84 + c), V(80 + c), ALU.mult, ALU.add
            )
        q2 = mv_d[1]
        wb = w_t.unsqueeze(1).broadcast_to([P, 4, F])
        nc.vector.tensor_mul(q2[:], qt[:], wb)
        nc.vector.tensor_add(
            ACC[:].rearrange("p c f -> p (c f)"),
            ACC[:].rearrange("p c f -> p (c f)"),
            q2[:].rearrange("p c f -> p (c f)"),
        )

        # ---- merge the chain accumulators into ACC (DVE) ----
        if N_R2 > 0:
            nc.vector.tensor_add(ACC[:], ACC[:], S0[:, :, 0:F])
            nc.vector.tensor_add(ACC[:], ACC[:], S1[:, :, 0:F])

        # ---- tree: 3 levels, halves-pairing (host shipped bit-reversed) ----
        # XY rows 0..3 = X_00,X_01,X_10,X_11; rows 4..7 = Y_00..Y_11
        # X_ij = A[i,0](first half) + B[0,j](second half)
        # Y_ij = A[i,1](first half) + B[1,j](second half)
        def level(src_m, XY, w_in, chunks=1):
            # src_m: [P, 4, w_in] fp16 (comp-major), returns [P, 4, w] view.
            # chunks=2 splits the columns so the DVE half of chunk k+1
            # overlaps the ACT exp/ln of chunk k.  exp stays f32 (fp16 exp
            # overflows past d~11); ln output is fp16 (softplus <= ~12) so
            # the final add runs at 2x.
            w = w_in // 2
            cw = w // chunks
            for ck in range(chunks):
                lo, hi = ck * cw, (ck + 1) * cw
                a = src_m[:, :, lo:hi]
                b = src_m[:, :, w + lo:w + hi]

                def bc2(apc):  # [P, cw] -> [P, 2, cw] broadcast over j
                    return apc.unsqueeze(1).broadcast_to([P, 2, cw])

                nc.vector.tensor_add(
                    XY[:, 0:2, lo:hi], bc2(a[:, 0, :]), b[:, 0:2, :])
                nc.vector.tensor_add(
                    XY[:, 2:4, lo:hi], bc2(a[:, 2, :]), b[:, 0:2, :])
                nc.vector.tensor_add(
                    XY[:, 4:6, lo:hi], bc2(a[:, 1, :]), b[:, 2:4, :])
                nc.vector.tensor_add(
                    XY[:, 6:8, lo:hi], bc2(a[:, 3, :]), b[:, 2:4, :])
                xv = XY[:, 0:4, lo:hi]
                yv = XY[:, 4:8, lo:hi]
                sp = SPa[:, :, lo:hi]
                spl = SPL[:, :, lo:hi]
                nc.vector.tensor_sub(yv, yv, xv)
                nc.scalar.activation(sp, yv, AF.Exp)
                nc.scalar.activation(spl, sp, AF.Ln, bias=1.0)
                nc.vector.tensor_add(xv, xv, spl)
            return XY[:, 0:4, :]

        goldp = pool.tile([P, 1], dt.float32, tag="goldp", name="goldp")
        if EN_TREE:
            m1 = level(ACC[:], XY1, F, chunks=2)

        # ---- gold: in-place predicated selects on the freed ACC comps ----
        # g = M[labp, lab]: ACC0 <- lab ? ACC1 : ACC0 ; ACC2 <- lab ? ACC3
        # : ACC2 ; ACC0 <- labp ? ACC2 : ACC0 ; reduce.  (CopyPredicated is
        # DVE-only on TRN2.)
        if EN_GOLD:
            nc.vector.copy_predicated(ACC[:, 0, :], lab16, ACC[:, 1, :])
            nc.vector.copy_predicated(ACC[:, 2, :], lab16, ACC[:, 3, :])
            nc.vector.copy_predicated(ACC[:, 0, :], labp16, ACC[:, 2, :])
        nc.vector.tensor_scalar(
            mv_d[0][:, 0, :], ACC[:, 0, :], 0.0, None, ALU.add, ALU.add,
            accum_out=goldp[:],
        )

        if EN_TREE and EN_LEV23:
            m3 = level(m1, XY2, H, chunks=2)
        elif EN_TREE:
            m3 = m1[:, :, 0:WSTOP]
        else:
            m3 = ACC[:, :, 0:WSTOP]

        # ---- store ----
        nc.sync.dma_start(outm_d[:], m3)
        nc.sync.dma_start(outg_d[:], goldp[:])

    nc.compile()

    # Both Exp and Ln live in the 'natural_log_exp_and_others' ACT table set,
    # but insert_act_table_loads picks the first set containing each function,
    # emitting an alternating exp/ln reload (1.3 us each) per tree level.
    # Retarget every load to the combined set and drop the now-redundant ones
    # (none carry sync_info).
    from concourse.hw_specs import get_activation_tables

    tables = list(get_activation_tables(nc.m.arch).keys())
    combined = tables.index("natural_log_exp_and_others")
    for b in nc.bb_map.values():
        insts = b.bb.instructions
        kept = []
        seen_load = False
        for ins in insts:
            if ins.opcode == "LoadActFuncSet":
                si = ins.sync_info
                assert not (si and (si.on_wait or si.on_update)), ins.name
                if seen_load:
                    continue
                ins.act_func_set_id = combined
                seen_load = True
            kept.append(ins)
        if len(kept) != len(insts):
            b.bb.instructions = kept
    return nc


def _get_nc():
    global _NC_CACHE
    if _NC_CACHE is None:
        _NC_CACHE = _build_nc()
    return _NC_CACHE


def _bitrev_perm(n):
    bits = n.bit_length() - 1
    idx = np.arange(n)
    rev = np.zeros(n, dtype=np.int64)
    for b in range(bits):
        rev |= ((idx >> b) & 1) << (bits - 1 - b)
    return rev


def kernel(**inputs):
    em = np.asarray(inputs["emission_scores"], dtype=np.float32)
    lab = np.asarray(inputs["label"]).astype(np.float32)
    w = np.asarray(inputs["who2who_state"]).astype(np.float32)
    p = np.asarray(inputs["position_state"]).astype(np.float32)
    w2w = np.asarray(inputs["who2who_params"], dtype=np.float32)
    pos = np.asarray(inputs["position_params"], dtype=np.float32)
    assert em.shape == (T, 2), em.shape

    labp = np.empty_like(lab)
    labp[0] = 0.0
    labp[1:] = lab[:-1]

    # per-partition streams in bit-reversed step order (tree pairs halves)
    rev = _bitrev_perm(F)

    def shape_stream(a16):
        return np.ascontiguousarray(
            a16.reshape(NCORES, P, F)[:, :, rev]
        )

    # global shift B > 0 so every shifted position entry W = V + B is
    # strictly positive (the ACT route builds masks as Relu(W - W*t)); the
    # pad class (19) becomes W = B.  Every element gets exactly one +B from
    # its position class, cancelled by shipping emissions as em - B.
    B = float(max(0.0, -pos.min()) + 1.0) if N_ACT > 0 else 0.0
    W_pos = np.zeros((20, 4), dtype=np.float64)
    W_pos[:19] = pos.reshape(19, 4)
    W_pos += B

    def dither(vals):
        # [n] f64 -> [P, n] f32 of fp16-representable values whose
        # per-partition mix averages to vals (Bresenham spread), killing
        # the fixed fp16 mask-value rounding bias in the gold sum
        vals = np.asarray(vals, np.float64).reshape(-1)
        lo16 = vals.astype(np.float16)
        lo = lo16.astype(np.float64)
        step = np.spacing(lo16).astype(np.float64)
        step = np.where(vals >= lo, step, -step)
        hi = lo + step
        frac = np.where(step != 0, (vals - lo) / np.where(step == 0, 1, step), 0)
        cnt = np.round(frac * P).astype(np.int64)  # partitions using hi
        q = np.arange(P)[:, None]
        use_hi = (q * cnt[None, :]) % P < cnt[None, :]
        return np.where(use_hi, hi[None, :], lo[None, :]).astype(np.float32)

    # who2who quadratic val_c(w) = a_c + b_c*w + c_c*w^2 through
    # (0, V0), (1, V1), (2, 0); a_c folds into the emission planes.
    # b,c are chosen per partition so the fp16-computed path yields
    # exactly-representable d1 = V1-V0 at w=1 and 2*fp16(-V0/2) at w=2.
    V0 = w2w.reshape(2, 4)[0].astype(np.float64)
    V1 = w2w.reshape(2, 4)[1].astype(np.float64)
    d1p = dither(V1 - V0).astype(np.float64)          # [P, 4]
    d2h = dither(-V0 / 2.0).astype(np.float64)        # [P, 4]
    qb = (2.0 * d1p - d2h).astype(np.float32)
    qc = (d2h - d1p).astype(np.float32)
    par16 = np.zeros((P, NPAR), dtype=np.float32)
    par16[:, 0:80] = dither(W_pos.reshape(-1))
    par16[:, 80:84] = qb
    par16[:, 84:88] = qc
    par16[:, 88] = -B
    par16[:, 89:169] = -par16[:, 0:80]
    par16 = np.ascontiguousarray(par16).view(np.float16)
    p16 = shape_stream(p.astype(np.float16))
    w16 = shape_stream(w.astype(np.float16))
    lab16 = shape_stream(lab.astype(np.int16).view(np.float16))
    labp16 = shape_stream(labp.astype(np.int16).view(np.float16))
    em16 = em.astype(np.float64).reshape(NCORES, P, F, 2)[:, :, rev, :]
    emP = [
        np.ascontiguousarray(
            (em16[..., c & 1] - B + V0[c]).astype(np.float16))
        for c in range(4)
    ]

    in_maps = []
    for k in range(NCORES):
        blob0 = np.concatenate(
            [par16, p16[k], w16[k], emP[0][k], emP[1][k], emP[2][k],
             emP[3][k], lab16[k], labp16[k]],
            axis=1,
        )
        in_maps.append({"blob0": np.ascontiguousarray(blob0)})

    nc = _get_nc()
    kr = bass_utils.run_bass_kernel_spmd(nc, in_maps, core_ids=list(range(NCORES)))
    global LAST_RESULTS
    LAST_RESULTS = kr
    results = kr.results

    # ---- host combine ----
    # outm: [P, 4*WSTOP] fp16, position i holds the product over the 8-step
    # block bitrev8(i) of its partition chunk; chunks ordered by (core, part).
    rev8 = _bitrev_perm(WSTOP)
    mats = np.empty((NCORES, P, WSTOP, 2, 2), dtype=np.float64)
    gold = 0.0
    for k, r in enumerate(results):
        m = np.asarray(r["outm"]).reshape(P, 4, WSTOP).astype(np.float64)
        mats[k] = m[:, :, rev8].transpose(0, 2, 1).reshape(P, WSTOP, 2, 2)
        gold += np.asarray(r["outg"], dtype=np.float64).sum()

    chain = mats.reshape(-1, 2, 2)
    while chain.shape[0] > 1:
        A = chain[0::2]
        B = chain[1::2]
        chain = np.logaddexp(
            A[:, :, 0:1] + B[:, 0:1, :], A[:, :, 1:2] + B[:, 1:2, :]
        )
    U = chain[0]
    total = np.logaddexp.reduce(U.reshape(-1))
    return np.stack([gold, total]).astype(np.float32)


if __name__ == "__main__":
    rng = np.random.default_rng(0)
    demo = dict(
        emission_scores=rng.standard_normal((T, 2)).astype(np.float32),
        label=rng.integers(0, 2, T),
        who2who_state=np.concatenate([[2], rng.integers(0, 2, T - 1)]),
        position_state=np.concatenate([[19], rng.integers(0, 19, T - 1)]),
        who2who_params=rng.standard_normal((2, 2, 2)).astype(np.float32),
        position_params=rng.standard_normal((19, 2, 2)).astype(np.float32),
    )
    print(kernel(**demo))


# revision 65
# speedup vs baseline: 1.0060x; 1.0060x over previous
"""Trainium2 Bass kernel for a 2-state linear-chain CRF loss (BiLSTM-CRF loss_fn).

Computes, for a single conversation of length T = 2,097,152:
  gold_score  = sum_t em[t, lab[t]] + sum_{t>0} trans[t][lab[t-1], lab[t]]
  total_score = logsumexp of the CRF forward recursion
where trans[t] = who2who_sub[w[t]] + position_sub[p[t]] (19 position + 2
who2who matrices; indices 19/2 select an all-zero padding matrix).

Design (one NeuronCore per contiguous chunk of 262,144 steps, 8 cores):

* Per-step matrices M[t][i,j] = trans[t][i,j] + em[t][j]:
  - ACC init = four emission planes (ACT Copy), with the who2who w=0
    matrix folded into them on the host, so neither costs anything extra.
  - 19 position classes by masked accumulation: each (class, comp) is one
    fused fp16 tensor_scalar mv = (p == c) * V (fast 4x 2-byte DVE mode).
    The accumulation adds -- the expensive half, 2x at best -- are split
    across sinks that run in parallel: N_R2 classes ride SBUF->SBUF
    *accumulate DMAs* (software-DGE cce add, otherwise-idle DMA engines)
    into two subaccumulator chains whose first writes are plain ts stores
    (two more classes with zero adds); the remaining classes accumulate
    into ACC on DVE with one batched [P,4F] add per class.  Chain tiles
    are row-padded so DMA descriptors stay at 4KB per partition (16KB
    single-descriptor Pool DMAs die at runtime on this stack).
  - who2who needs no masks at all: val_c(w) = a_c + b_c*w + c_c*w^2 fits
    the three states exactly (pad w=2 -> 0 by construction); a_c lives in
    the emission planes, and b,c are built per partition from
    fp16-representable targets so the computed path rounds exactly.
  - every shipped table value is per-partition DITHERED between its two
    neighboring fp16 values (Bresenham mix matching the true value), so
    the fixed fp16 rounding of each (class, comp) cell averages out of the
    gold sum instead of biasing it (~3e-3 -> ~6e-4 rel).

* Gold score: gold = sum_t M[t][lab[t-1], lab[t]] via three in-place
  copy_predicated selects (int16 lab/labp as masks) on the finished ACC
  comps -- legal because the tree's level-1 reads happen first -- plus one
  fused accum_out reduce.  ~10x cheaper than exact per-cell counting.

* Forward pass: the recursion is a product of 2x2 matrices in the (log,+)
  semiring (associative).  The host ships every per-partition stream in
  BIT-REVERSED step order, so both on-device tree levels combine the first
  half of a tile with the second half -- fully contiguous fp16 operands at
  2x, no strided gathers.  LSE(a,b) = a + ln(1+exp(b-a)): exp to f32 PSUM
  (fp16 exp overflows past d~11), ln back to fp16 so the final add runs at
  2x; column-chunked so DVE work overlaps ACT.  The device stops at 512
  matrices per partition (4 source steps each; 3/4 of all tree combines);
  the host finishes the remaining 19 tree levels vectorized in numpy --
  O(cores * P * WSTOP) work, independent of T.

* All inputs ship as one per-core fp16 blob
  [par | p | w | emP0..emP3 | lab | labp] split into four DMAs (par+p
  first so mask work starts early; labels last, mid-chain, for gold).

The cost-model timeline: DVE ~103us busy (the 4x tensor_scalar mask work
is irreducible there), DMA engines ~81us, Pool ~14us (descriptor gen),
ACT ~21us; wall 120.6us vs 249.1us for the counting-based baseline.

Accuracy vs the fp32 jax reference: gold ~6e-4 rel; total ~1.1e-3 rel (the
reference's own sequential-fp32-scan rounding wander at T=2M; a float64
ground truth sits on our side of it).
"""

from contextlib import ExitStack

import numpy as np

import concourse.bass as bass
import concourse.bacc as bacc
import concourse.mybir as mybir
import concourse.tile as tile
from concourse import bass_utils

dt = mybir.dt
ALU = mybir.AluOpType
AF = mybir.ActivationFunctionType
AX = mybir.AxisListType

T = 2097152
NCORES = 8
P = 128                  # SBUF partitions
L = T // NCORES          # steps per core = 262144
F = L // P               # steps per partition = 2048
H = F // 2               # pairs per partition at tree level 1
WSTOP = 512              # matrices per partition shipped to the host
NPOS = 19                # position classes with nonzero matrices
# param row layout (f32 columns):
#   0..79    W_pos  = pos_param + B for 20 classes (incl. pad 19 -> B)
#   80..87   V_w2w  (unshifted)
#   88       -B     (em-init bias; cancels the one +B every element gets)
#   89..168  -W_pos (ACT Relu scale operands)
NPAR = 169
# blob (fp16): [par | p | w | emP0..emP3 | lab | labp]; emPc = em_{c&1} - B
# + a_c with a_c the constant term of the who2who quadratic for comp c
W0 = 8 * F + 2 * NPAR

# class-sum routing: 19 position classes (who2who is handled by an exact
# quadratic in w).  N_R2 classes ride the DMA-accumulate chains; the next
# two write their mv directly into the chain accumulators S0/S1 (zero
# adds); the rest accumulate into ACC on DVE with one batched whole-class
# add each.  N_ACT>0 would route chain classes through a shifted-Relu mask
# build on the ACT engine (measured slower; kept for reference).
N_R2 = 12
N_ACT = 0
MV4_BUFS = 3

# debug switches (bisect aids; all True for the real kernel)
EN_TREE = True
EN_GOLD = True
EN_LEV23 = True

_NC_CACHE = None
LAST_RESULTS = None  # BassKernelResults of the most recent kernel() call


def _comp(i, j):
    return i * 2 + j


def _build_nc():
    nc = bacc.Bacc()

    b0_d = nc.dram_tensor("blob0", [P, W0], dt.float16, kind="ExternalInput")
    outm_d = nc.dram_tensor("outm", [P, 4 * WSTOP], dt.float16,
                            kind="ExternalOutput")
    outg_d = nc.dram_tensor("outg", [P, 1], dt.float32, kind="ExternalOutput")

    # const APs for the ACT-route Square bias values (-class id); the
    # ACT-fed classes sit at odd chain positions
    for _v in sorted({-float(2 * k + 1) for k in range(N_ACT)}):
        if (dt.float32, _v) in nc.const_aps.aps:
            continue
        _t = nc.alloc_sbuf_tensor(f"const-float32-{_v}", [128, 1], dt.float32)
        nc.gpsimd.memset(_t.ap(), _v)
        nc.const_aps.aps[(dt.float32, _v)] = _t.ap()
    nc.all_engine_barrier()

    with ExitStack() as ctx:
        tc = ctx.enter_context(tile.TileContext(nc))
        pool = ctx.enter_context(tc.tile_pool(name="main", bufs=1))
        ppool = ctx.enter_context(tc.psum_pool(name="psum", bufs=1))

        # ---- loads ----
        # blob layout (fp16 cols): [par | p | w | em0 | em1 | lab | labp]
        b0 = pool.tile([P, W0], dt.float16, tag="b0", name="b0")
        parw = 2 * NPAR
        o_p = parw
        o_w = o_p + F
        o_e0 = o_w + F
        o_lab = o_e0 + 4 * F
        o_labp = o_lab + F
        # head1: par|p|w (needed by every mask op), head2: emissions,
        # tail: labels (needed only by gold, emitted mid-chain below)
        nc.sync.dma_start(b0[:, 0:o_w], b0_d[:, 0:o_w])
        nc.sync.dma_start(b0[:, o_w:o_e0], b0_d[:, o_w:o_e0])
        nc.sync.dma_start(b0[:, o_e0:o_lab], b0_d[:, o_e0:o_lab])

        par32 = b0[:, 0:parw].bitcast(dt.float32)
        p_t = b0[:, o_p:o_p + F]
        w_t = b0[:, o_w:o_w + F]
        emP = [b0[:, o_e0 + c * F:o_e0 + (c + 1) * F] for c in range(4)]
        # labels ship as int16 {0,1} in the fp16 blob slots (CopyPredicated
        # requires an integer mask dtype)
        lab16 = b0[:, o_lab:o_lab + F].bitcast(dt.int16)
        labp16 = b0[:, o_labp:o_labp + F].bitcast(dt.int16)

        def V(col):
            return par32[:, col:col + 1]

        # ---- accumulators ----
        # ACC  : DVE-adds sink, init = emission planes (M[i,j] = trans+em[j])
        # S0/S1: DMA-accumulate chains, init = one direct-write class each
        # S/mv4 rows are padded to F+8 so the comp dim cannot merge with the
        # row dim during DMA lowering: per-partition contiguous descriptor
        # chunks stay at 4KB (16KB single-descriptor Pool DMAs fail at
        # runtime on this stack).
        FP = F + 8
        ACC = pool.tile([P, 4, F], dt.float16, tag="ACC", name="ACC")
        S0 = (pool.tile([P, 4, FP], dt.float16, tag="S0", name="S0")
              if N_R2 > 0 else None)
        S1 = (pool.tile([P, 4, FP], dt.float16, tag="S1", name="S1")
              if N_R2 > 0 else None)
        # tree tiles allocated up front; XY1 doubles as a class-phase mv
        # buffer (idle until the tree) and SPa lives in PSUM
        XY1 = pool.tile([P, 8, H], dt.float16, tag="XY1", name="XY1")
        XY2 = pool.tile([P, 8, H // 2], dt.float16, tag="XY2", name="XY2")
        SPa = ppool.tile([P, 4, H], dt.float32, tag="SPa", name="SPa")
        SPL = pool.tile([P, 4, H], dt.float16, tag="SPL", name="SPL")
        # emission planes ship pre-shifted (em - B + a_c) on the host, so
        # plain copies init ACC
        for c in range(4):
            nc.scalar.activation(ACC[:, c, :], emP[c], AF.Copy)

        # classes: (src, cval, vcol). position classes only (incl. pad iff
        # the shifted ACT route is active); who2who is handled by the exact
        # quadratic in w below (pad w=2 -> 0 by construction).
        npos_cls = NPOS + (1 if N_ACT > 0 else 0)
        classes = [(p_t, float(c), 4 * c) for c in range(npos_cls)]
        r2_classes = classes[:N_R2]
        direct_classes = classes[N_R2:N_R2 + 2]  # init S0 and S1 (no adds)
        rest_classes = classes[N_R2 + 2:]

        mv4 = [
            pool.tile([P, 4, FP], dt.float16, tag=f"mv4_{i}", name=f"mv4_{i}")
            for i in range(MV4_BUFS if N_R2 > 0 else 0)
        ]
        mv_d = [
            pool.tile([P, 4, F], dt.float16, tag="mvd0", name="mvd0"),
            pool.tile([P, 4, F], dt.float16, tag="mvd1", name="mvd1"),
        ]

        # direct-write classes: their mv IS the chain init (no adds at
        # all), so every chain hop accumulates instead of copying
        for S, (src, cval, col) in zip((S0, S1), direct_classes):
            for c in range(4):
                nc.vector.tensor_scalar(
                    S[:, c, 0:F], src, cval, V(col + c),
                    ALU.is_equal, ALU.mult,
                )

        dve_cls = rest_classes

        def emit_dve_cls(k):
            src, cval, col = dve_cls[k]
            m = mv_d[k % 2]
            for c in range(4):
                nc.vector.tensor_scalar(
                    m[:, c, :], src, cval, V(col + c), ALU.is_equal, ALU.mult
                )
            nc.vector.tensor_add(
                ACC[:].rearrange("p c f -> p (c f)"),
                ACC[:].rearrange("p c f -> p (c f)"),
                m[:].rearrange("p c f -> p (c f)"),
            )

        # Interleave: per R2 class emit its 4 fused-ts + one chain hop, then
        # a pro-rata slice of the GPSIMD classes (keeps the Pool queue
        # alternating swdge preps with adds so the chains never starve) and
        # of the DVE classes (keeps DVE busy while the chains drain).
        n_dv = len(dve_cls)
        dv_k = 0
        tsq = [
            pool.tile([P, F], dt.float16, tag=f"tsq{i}", name=f"tsq{i}")
            for i in range(2 if N_ACT > 0 else 0)
        ]
        # ACT-fed chain classes sit at odd positions so the DVE-fed hops
        # interleave and the chains never wait on the slower ACT producer
        act_ri = {2 * k + 1 for k in range(N_ACT)}
        for ri, (src, cval, col) in enumerate(r2_classes):
            m4 = mv4[ri % MV4_BUFS]
            if ri in act_ri:
                # ACT-built mv: t = (p-c)^2 ; mv_k = Relu(W_k - W_k*t)
                t = tsq[ri % 2]
                nc.scalar.activation(t[:], src, AF.Square, bias=-cval)
                for c in range(4):
                    nc.scalar.activation(
                        m4[:, c, 0:F], t[:], AF.Relu,
                        bias=V(col + c), scale=V(89 + col + c),
                    )
            else:
                for c in range(4):
                    nc.vector.tensor_scalar(
                        m4[:, c, 0:F], src, cval, V(col + c),
                        ALU.is_equal, ALU.mult,
                    )
            S = S0 if ri % 2 == 0 else S1
            nc.gpsimd.dma_start(S[:, :, 0:F], m4[:, :, 0:F],
                                accum_op=ALU.add)
            for _ in range((n_dv * (ri + 1)) // N_R2 - (n_dv * ri) // N_R2):
                emit_dve_cls(dv_k)
                dv_k += 1
            if ri == 5:
                # labels, needed late (gold) -- emitted here so the head
                # DMAs and early chain hops aren't delayed
                nc.sync.dma_start(b0[:, o_lab:W0], b0_d[:, o_lab:W0])
        while dv_k < n_dv:
            emit_dve_cls(dv_k)
            dv_k += 1

        # ---- who2who via the exact quadratic a_c + b_c*w + c_c*w^2 ----
        # (a_c folded into the emission planes; b at cols 80..83, c at 84..87)
        qt = mv_d[0]
        for c in range(4):
            nc.vector.tensor_scalar(
                qt[:, c, :], w_t, V(84 + c), V(80 + c), ALU.mult, ALU.add
            )
        q2 = mv_d[1]
        wb = w_t.unsqueeze(1).broadcast_to([P, 4, F])
        nc.vector.tensor_mul(q2[:], qt[:], wb)
        nc.vector.tensor_add(
            ACC[:].rearrange("p c f -> p (c f)"),
            ACC[:].rearrange("p c f -> p (c f)"),
            q2[:].rearrange("p c f -> p (c f)"),
        )

        # ---- merge the chain accumulators into ACC (DVE) ----
        if N_R2 > 0:
            nc.vector.tensor_add(ACC[:], ACC[:], S0[:, :, 0:F])
            nc.vector.tensor_add(ACC[:], ACC[:], S1[:, :, 0:F])

        # ---- tree: 3 levels, halves-pairing (host shipped bit-reversed) ----
        # XY rows 0..3 = X_00,X_01,X_10,X_11; rows 4..7 = Y_00..Y_11
        # X_ij = A[i,0](first half) + B[0,j](second half)
        # Y_ij = A[i,1](first half) + B[1,j](second half)
        def level(src_m, XY, w_in, chunks=1):
            # src_m: [P, 4, w_in] fp16 (comp-major), returns [P, 4, w] view.
            # chunks=2 splits the columns so the DVE half of chunk k+1
            # overlaps the ACT exp/ln of chunk k.  exp stays f32 (fp16 exp
            # overflows past d~11); ln output is fp16 (softplus <= ~12) so
            # the final add runs at 2x.
            w = w_in // 2
            cw = w // chunks
            for ck in range(chunks):
                lo, hi = ck * cw, (ck + 1) * cw
                a = src_m[:, :, lo:hi]
                b = src_m[:, :, w + lo:w + hi]

                def bc2(apc):  # [P, cw] -> [P, 2, cw] broadcast over j
                    return apc.unsqueeze(1).broadcast_to([P, 2, cw])

                nc.vector.tensor_add(
                    XY[:, 0:2, lo:hi], bc2(a[:, 0, :]), b[:, 0:2, :])
                nc.vector.tensor_add(
                    XY[:, 2:4, lo:hi], bc2(a[:, 2, :]), b[:, 0:2, :])
                nc.vector.tensor_add(
                    XY[:, 4:6, lo:hi], bc2(a[:, 1, :]), b[:, 2:4, :])
                nc.vector.tensor_add(
                    XY[:, 6:8, lo:hi], bc2(a[:, 3, :]), b[:, 2:4, :])
                xv = XY[:, 0:4, lo:hi]
                yv = XY[:, 4:8, lo:hi]
                sp = SPa[:, :, lo:hi]
                spl = SPL[:, :, lo:hi]
                nc.vector.tensor_sub(yv, yv, xv)
                nc.scalar.activation(sp, yv, AF.Exp)
                nc.scalar.activation(spl, sp, AF.Ln, bias=1.0)
                nc.vector.tensor_add(xv, xv, spl)
            return XY[:, 0:4, :]

        goldp = pool.tile([P, 1], dt.float32, tag="goldp", name="goldp")
        if EN_TREE:
            m1 = level(ACC[:], XY1, F, chunks=2)

        # ---- gold: in-place predicated selects on the freed ACC comps ----
        # g = M[labp, lab]: ACC0 <- lab ? ACC1 : ACC0 ; ACC2 <- lab ? ACC3
        # : ACC2 ; ACC0 <- labp ? ACC2 : ACC0 ; reduce.  (CopyPredicated is
        # DVE-only on TRN2.)
        if EN_GOLD:
            nc.vector.copy_predicated(ACC[:, 0, :], lab16, ACC[:, 1, :])
            nc.vector.copy_predicated(ACC[:, 2, :], lab16, ACC[:, 3, :])
            nc.vector.copy_predicated(ACC[:, 0, :], labp16, ACC[:, 2, :])
        nc.vector.tensor_scalar(
            mv_d[0][:, 0, :], ACC[:, 0, :], 0.0, None, ALU.add, ALU.add,
            accum_out=goldp[:],
        )

        if EN_TREE and EN_LEV23:
            m3 = level(m1, XY2, H, chunks=2)
        elif EN_TREE:
            m3 = m1[:, :, 0:WSTOP]
        else:
            m3 = ACC[:, :, 0:WSTOP]

        # ---- store (two column chunks; chunk 0 fires as soon as the
        # tree's first column chunk lands) ----
        wq = WSTOP // 2
        od = outm_d[:].rearrange("p (c w) -> p c w", c=4)
        nc.sync.dma_start(od[:, :, 0:wq], m3[:, :, 0:wq])
        nc.sync.dma_start(od[:, :, wq:WSTOP], m3[:, :, wq:WSTOP])
        nc.sync.dma_start(outg_d[:], goldp[:])

    nc.compile()

    # Both Exp and Ln live in the 'natural_log_exp_and_others' ACT table set,
    # but insert_act_table_loads picks the first set containing each function,
    # emitting an alternating exp/ln reload (1.3 us each) per tree level.
    # Retarget every load to the combined set and drop the now-redundant ones
    # (none carry sync_info).
    from concourse.hw_specs import get_activation_tables

    tables = list(get_activation_tables(nc.m.arch).keys())
    combined = tables.index("natural_log_exp_and_others")
    for b in nc.bb_map.values():
        insts = b.bb.instructions
        kept = []
        seen_load = False
        for ins in insts:
            if ins.opcode == "LoadActFuncSet":
                si = ins.sync_info
                assert not (si and (si.on_wait or si.on_update)), ins.name
                if seen_load:
                    continue
                ins.act_func_set_id = combined
                seen_load = True
            kept.append(ins)
        if len(kept) != len(insts):
            b.bb.instructions = kept
    return nc


def _get_nc():
    global _NC_CACHE
    if _NC_CACHE is None:
        _NC_CACHE = _build_nc()
    return _NC_CACHE


def _bitrev_perm(n):
    bits = n.bit_length() - 1
    idx = np.arange(n)
    rev = np.zeros(n, dtype=np.int64)
    for b in range(bits):
        rev |= ((idx >> b) & 1) << (bits - 1 - b)
    return rev


def kernel(**inputs):
    em = np.asarray(inputs["emission_scores"], dtype=np.float32)
    lab = np.asarray(inputs["label"]).astype(np.float32)
    w = np.asarray(inputs["who2who_state"]).astype(np.float32)
    p = np.asarray(inputs["position_state"]).astype(np.float32)
    w2w = np.asarray(inputs["who2who_params"], dtype=np.float32)
    pos = np.asarray(inputs["position_params"], dtype=np.float32)
    assert em.shape == (T, 2), em.shape

    labp = np.empty_like(lab)
    labp[0] = 0.0
    labp[1:] = lab[:-1]

    # per-partition streams in bit-reversed step order (tree pairs halves)
    rev = _bitrev_perm(F)

    def shape_stream(a16):
        return np.ascontiguousarray(
            a16.reshape(NCORES, P, F)[:, :, rev]
        )

    # global shift B > 0 so every shifted position entry W = V + B is
    # strictly positive (the ACT route builds masks as Relu(W - W*t)); the
    # pad class (19) becomes W = B.  Every element gets exactly one +B from
    # its position class, cancelled by shipping emissions as em - B.
    B = float(max(0.0, -pos.min()) + 1.0) if N_ACT > 0 else 0.0
    W_pos = np.zeros((20, 4), dtype=np.float64)
    W_pos[:19] = pos.reshape(19, 4)
    W_pos += B

    def dither(vals):
        # [n] f64 -> [P, n] f32 of fp16-representable values whose
        # per-partition mix averages to vals (Bresenham spread), killing
        # the fixed fp16 mask-value rounding bias in the gold sum
        vals = np.asarray(vals, np.float64).reshape(-1)
        lo16 = vals.astype(np.float16)
        lo = lo16.astype(np.float64)
        step = np.spacing(lo16).astype(np.float64)
        step = np.where(vals >= lo, step, -step)
        hi = lo + step
        frac = np.where(step != 0, (vals - lo) / np.where(step == 0, 1, step), 0)
        cnt = np.round(frac * P).astype(np.int64)  # partitions using hi
        q = np.arange(P)[:, None]
        use_hi = (q * cnt[None, :]) % P < cnt[None, :]
        return np.where(use_hi, hi[None, :], lo[None, :]).astype(np.float32)

    # who2who quadratic val_c(w) = a_c + b_c*w + c_c*w^2 through
    # (0, V0), (1, V1), (2, 0); a_c folds into the emission planes.
    # b,c are chosen per partition so the fp16-computed path yields
    # exactly-representable d1 = V1-V0 at w=1 and 2*fp16(-V0/2) at w=2.
    V0 = w2w.reshape(2, 4)[0].astype(np.float64)
    V1 = w2w.reshape(2, 4)[1].astype(np.float64)
    d1p = dither(V1 - V0).astype(np.float64)          # [P, 4]
    d2h = dither(-V0 / 2.0).astype(np.float64)        # [P, 4]
    qb = (2.0 * d1p - d2h).astype(np.float32)
    qc = (d2h - d1p).astype(np.float32)
    par16 = np.zeros((P, NPAR), dtype=np.float32)
    par16[:, 0:80] = dither(W_pos.reshape(-1))
    par16[:, 80:84] = qb
    par16[:, 84:88] = qc
    par16[:, 88] = -B
    par16[:, 89:169] = -par16[:, 0:80]
    par16 = np.ascontiguousarray(par16).view(np.float16)
    p16 = shape_stream(p.astype(np.float16))
    w16 = shape_stream(w.astype(np.float16))
    lab16 = shape_stream(lab.astype(np.int16).view(np.float16))
    labp16 = shape_stream(labp.astype(np.int16).view(np.float16))
    em16 = em.astype(np.float64).reshape(NCORES, P, F, 2)[:, :, rev, :]
    emP = [
        np.ascontiguousarray(
            (em16[..., c & 1] - B + V0[c]).astype(np.float16))
        for c in range(4)
    ]

    in_maps = []
    for k in range(NCORES):
        blob0 = np.concatenate(
            [par16, p16[k], w16[k], emP[0][k], emP[1][k], emP[2][k],
             emP[3][k], lab16[k], labp16[k]],
            axis=1,
        )
        in_maps.append({"blob0": np.ascontiguousarray(blob0)})

    nc = _get_nc()
    kr = bass_utils.run_bass_kernel_spmd(nc, in_maps, core_ids=list(range(NCORES)))
    global LAST_RESULTS
    LAST_RESULTS = kr
    results = kr.results

    # ---- host combine ----
    # outm: [P, 4*WSTOP] fp16, position i holds the product over the 8-step
    # block bitrev8(i) of its partition chunk; chunks ordered by (core, part).
    rev8 = _bitrev_perm(WSTOP)
    mats = np.empty((NCORES, P, WSTOP, 2, 2), dtype=np.float64)
    gold = 0.0
    for k, r in enumerate(results):
        m = np.asarray(r["outm"]).reshape(P, 4, WSTOP).astype(np.float64)
        mats[k] = m[:, :, rev8].transpose(0, 2, 1).reshape(P, WSTOP, 2, 2)
        gold += np.asarray(r["outg"], dtype=np.float64).sum()

    chain = mats.reshape(-1, 2, 2)
    while chain.shape[0] > 1:
        A = chain[0::2]
        B = chain[1::2]
        chain = np.logaddexp(
            A[:, :, 0:1] + B[:, 0:1, :], A[:, :, 1:2] + B[:, 1:2, :]
        )
    U = chain[0]
    total = np.logaddexp.reduce(U.reshape(-1))
    return np.stack([gold, total]).astype(np.float32)


if __name__ == "__main__":
    rng = np.random.default_rng(0)
    demo = dict(
        emission_scores=rng.standard_normal((T, 2)).astype(np.float32),
        label=rng.integers(0, 2, T),
        who2who_state=np.concatenate([[2], rng.integers(0, 2, T - 1)]),
        position_state=np.concatenate([[19], rng.integers(0, 19, T - 1)]),
        who2who_params=rng.standard_normal((2, 2, 2)).astype(np.float32),
        position_params=rng.standard_normal((19, 2, 2)).astype(np.float32),
    )
    print(kernel(**demo))
